# revision 17
# baseline (speedup 1.0000x reference)
"""Multi-head attention (S=2048, D=1024, H=16, dk=dv=64) on 8 TRN2 NeuronCores.

Sharding: head-parallel tensor parallelism. Core c owns heads {2c, 2c+1}:
  - QT/KT [128, S] (two heads stacked on partitions); V via PE-transpose of
    VT, augmented with a ones column so the ctx matmul also produces the
    softmax denominators (softmax runs over the partition axis).
  - scoresT tiles -> exp on ACT (scale=1/8 folded in) -> ctx accumulation.
  - per s-chunk: normalize ctxT, AllGather the [128, chunk] block across
    cores -> [1024, chunk] concat, then a 128-column slice of the output
    projection per core (outT layout). Host unshard = concat + transpose.

Overlap structure: enc_k/enc_q stream on the two HWDGE queues while enc_v
cast-streams on the SWDGE queue, all from t=0. K and the first half of Q
project first (the minimum needed to start the exp stream); V and the
second Q half project inside the first chunk's scores loop using
time-shared PSUM pools. s is processed in three chunks (1024/512/512) so
the per-chunk ctx AllGathers start early and the last one is small.

Compute dtype: bf16 operands, fp32 PSUM accumulation, softmax in fp32.
"""

import numpy as np

import concourse.bass as bass
import concourse.mybir as mybir
import concourse.tile as tile
from concourse import bacc
from concourse.bass_utils import run_bass_kernel_spmd

S = 2048
D = 1024
H = 16
DK = 64
DV = 64
NCORES = 8
HPC = H // NCORES          # heads per core = 2
FW = HPC * DV              # per-core feature width = 128
P = 128                    # partitions
KT_D = D // P              # 8 contraction tiles over D
TT = S // P                # 16 tiles over t (keys)
NQ = 512                   # matmul moving free dim
CW = 512                   # s-chunk width (ctx/AG granularity)
NC_CH = S // CW            # 4 chunks
VA = 2 * (DV + 1)          # V_aug feature width

F32 = mybir.dt.float32
BF16 = mybir.dt.bfloat16
EXPF = mybir.ActivationFunctionType.Exp

_cache = {}


def _prep_w(w):
    """[D, FW] -> [128, KT_D*FW]: row p holds all d-tiles' row p."""
    return np.ascontiguousarray(
        np.transpose(w.reshape(KT_D, P, FW), (1, 0, 2)).reshape(P, KT_D * FW)
    )



def build():
    nc = bacc.Bacc(None, target_bir_lowering=False)

    enc_in = {
        x: nc.dram_tensor(f"enc{x}_t", [D, S], F32, kind="ExternalInput")
        for x in ("q", "k", "v")
    }
    w_in = {
        n: nc.dram_tensor(n, [P, KT_D * FW], F32, kind="ExternalInput")
        for n in ("wq", "wk", "wv", "wo")
    }
    out_t = nc.dram_tensor("outT", [FW, S], F32, kind="ExternalOutput")

    from concourse.bass import _add_dep_helper
    from concourse.masks import make_identity

    with tile.TileContext(nc) as tc:
        with (
            tc.tile_pool(name="wts", bufs=1) as wts,
            tc.tile_pool(name="encp", bufs=3) as encp,
            tc.tile_pool(name="qkv", bufs=1) as qkv,
            tc.tile_pool(name="expp", bufs=13) as expp,
            tc.tile_pool(name="catp", bufs=1) as catp,
            tc.tile_pool(name="catin", bufs=3) as catin,
            tc.tile_pool(name="misc", bufs=1) as misc,
            tc.tile_pool(name="dram", bufs=1, space="DRAM") as dram,
        ):
            rg = [list(range(NCORES))]

            # ---- weights: contiguous f32 DMA + DVE cast ----
            wtiles = {}
            for name in ("wq", "wk", "wv", "wo"):
                wraw = encp.tile(
                    [P, KT_D * FW], F32, tag="rawwt", bufs=2, name=name
                )
                nc.sync.dma_start(wraw[:], w_in[name][:])
                wt = wts.tile([P, KT_D, FW], BF16, tag=f"w_{name}", name=name)
                nc.vector.tensor_copy(
                    wt[:], wraw.rearrange("p (kt m) -> p kt m", kt=KT_D)
                )
                wtiles[name] = wt

            ident = wts.tile([P, P], BF16, tag="ident")
            make_identity(nc, ident)

            # persistent SBUF state
            qt_sb = qkv.tile([P, S], BF16, tag="qt")
            kt_sb = qkv.tile([P, S], BF16, tag="kt")
            vt_sb = qkv.tile([P, S], BF16, tag="vt")
            v_aug = qkv.tile([P, TT, VA], BF16, tag="vaug")
            cat_loc = catp.tile([P, S], BF16, tag="cat")
            nc.any.memset(v_aug[:, :, DV : DV + 1], 1.0)
            nc.any.memset(v_aug[:, :, 2 * DV + 1 : 2 * DV + 2], 1.0)

            # K tiles: halves on the two HWDGE queues, DVE cast
            def load_k(dt):
                raw = encp.tile([P, S], F32, tag="rawk", bufs=2, name="raw")
                nc.sync.dma_start(
                    raw[:, :1024], enc_in["k"][dt * P : (dt + 1) * P, 0:1024]
                )
                nc.scalar.dma_start(
                    raw[:, 1024:], enc_in["k"][dt * P : (dt + 1) * P, 1024:]
                )
                t = encp.tile([P, S], BF16, tag="bfk", bufs=3, name="bf")
                nc.vector.tensor_copy(t[:], raw[:])
                return t

            # Q quarter tiles [128, 512], alternating queues, DVE cast
            def load_qq(qq, dt):
                c0 = qq * CW
                raw = encp.tile([P, CW], F32, tag="rawq", bufs=4, name="raw")
                eng = nc.sync if dt % 2 == 0 else nc.scalar
                eng.dma_start(
                    raw[:], enc_in["q"][dt * P : (dt + 1) * P, c0 : c0 + CW]
                )
                t = encp.tile([P, CW], BF16, tag="bfq", bufs=8, name="bf")
                nc.vector.tensor_copy(t[:], raw[:])
                return t

            # ---- phase 0: K, Q quarters 0+1; PE warm-up burst ----
            ps_p_cm = tc.tile_pool(name="ps_p", bufs=1, space="PSUM")
            ps_p = ps_p_cm.__enter__()
            kacc = {
                sc4: ps_p.tile([P, NQ], F32, tag=f"ka{sc4}", name=f"ka{sc4}")
                for sc4 in range(4)
            }
            ek0 = load_k(0)
            wm = ps_p.tile([P, NQ], F32, tag="warm", name="wm")
            for _ in range(24):
                nc.tensor.matmul(
                    wm[:], wtiles["wk"][:, 0, :], ek0[:, 0:NQ],
                    start=True, stop=True,
                )
            for dt in range(KT_D):
                ek = ek0 if dt == 0 else load_k(dt)
                for sc4 in range(4):
                    nc.tensor.matmul(
                        kacc[sc4][:],
                        wtiles["wk"][:, dt, :],
                        ek[:, sc4 * NQ : (sc4 + 1) * NQ],
                        start=(dt == 0),
                        stop=(dt == KT_D - 1),
                    )
            for sc4 in range(4):
                nc.vector.tensor_copy(
                    kt_sb[:, sc4 * NQ : (sc4 + 1) * NQ], kacc[sc4][:]
                )
            qq01 = {
                qq: ps_p.tile([P, CW], F32, tag=f"qq{qq}", name=f"qq{qq}")
                for qq in range(2)
            }
            q1_dma0 = None
            for qq in range(2):
                for dt in range(KT_D):
                    eq = load_qq(qq, dt)
                    nc.tensor.matmul(
                        qq01[qq][:],
                        wtiles["wq"][:, dt, :],
                        eq[:],
                        start=(dt == 0),
                        stop=(dt == KT_D - 1),
                    )
                nc.vector.tensor_copy(
                    qt_sb[:, qq * CW : (qq + 1) * CW], qq01[qq][:]
                )
            ps_p_cm.__exit__(None, None, None)

            # ---- enc_v: SWDGE cast-DMA stream (own queue) ----
            ev_tiles = []
            for dt in range(KT_D):
                ev = encp.tile([P, S], BF16, tag="encv", bufs=6, name="ev")
                nc.gpsimd.dma_start(
                    ev[:], enc_in["v"][dt * P : (dt + 1) * P, :]
                )
                ev_tiles.append(ev)

            # ---- attention stream ----
            ps_at_cm = tc.tile_pool(name="ps_at", bufs=1, space="PSUM")
            ps_at = ps_at_cm.__enter__()
            ctx_ps = {}
            gas = {}

            def scores_tt(ci, tt):
                m = ps_at.tile([P, 1024], F32, tag="mega", bufs=2, name="m")
                s0 = ci * CW
                for h in range(HPC):
                    nc.tensor.matmul(
                        m[:, h * NQ : (h + 1) * NQ],
                        kt_sb[h * DK : (h + 1) * DK, tt * P : (tt + 1) * P],
                        qt_sb[h * DK : (h + 1) * DK, s0 : s0 + NQ],
                        start=True,
                        stop=True,
                    )
                ex = expp.tile(
                    [P, 1024], BF16, tag=f"exp{tt % 2}", bufs=13, name="ex"
                )
                nc.scalar.activation(ex[:], m[:], EXPF, scale=1.0 / np.sqrt(DK))
                return ex

            def ctx_op(ci, k, ex):
                for h in range(HPC):
                    nc.tensor.matmul(
                        ctx_ps[ci][h * DV + (h > 0) : (h + 1) * DV + 1, :]
                        if False
                        else ctx_ps[(ci, h)][:, :],
                        v_aug[:, k, h * (DV + 1) : (h + 1) * (DV + 1)],
                        ex[:, h * NQ : (h + 1) * NQ],
                        start=(k == 0),
                        stop=(k == TT - 1),
                    )

            def alloc_ctx(ci):
                for h in range(HPC):
                    ctx_ps[(ci, h)] = ps_cx.tile(
                        [DV + 1, CW], F32, tag=f"cx{h}", name=f"cx{ci}{h}"
                    )

            def normalize(ci):
                c0 = ci * CW
                for h in range(HPC):
                    den = misc.tile([1, CW], F32, tag="den", name="den")
                    nc.vector.tensor_copy(
                        den[:], ctx_ps[(ci, h)][DV : DV + 1, :]
                    )
                    recip = misc.tile([1, CW], F32, tag="recip", name="recip")
                    nc.vector.reciprocal_approx_fast(recip[:], den[:])
                    bcast = misc.tile([DV, CW], F32, tag="bcast", name="bcast")
                    nc.gpsimd.partition_broadcast(bcast[:], recip[:])
                    nc.vector.tensor_mul(
                        cat_loc[h * DV : (h + 1) * DV, c0 : c0 + CW],
                        ctx_ps[(ci, h)][0:DV, :],
                        bcast[:],
                    )
                cb = dram.tile([P, CW], BF16, tag=f"catb{ci}", name="cb")
                nc.sync.dma_start(cb[:], cat_loc[:, c0 : c0 + CW])
                ga = dram.tile([D, CW], BF16, tag=f"catall{ci}", name="ga")
                nc.gpsimd.collective_compute(
                    "AllGather",
                    mybir.AluOpType.bypass,
                    ins=[cb[:].opt()],
                    outs=[ga[:].opt()],
                    replica_groups=rg,
                )
                gas[ci] = ga

            def outproj(ci):
                c0 = ci * CW
                m = ps_at.tile([P, 1024], F32, tag="mega", bufs=2, name="om")
                for kt in range(KT_D):
                    ct = catin.tile([P, CW], BF16, tag="catkt", name="ct")
                    nc.sync.dma_start(
                        ct[:], gas[ci][kt * P : (kt + 1) * P, :]
                    )
                    nc.tensor.matmul(
                        m[:, 0:CW],
                        wtiles["wo"][:, kt, :],
                        ct[:],
                        start=(kt == 0),
                        stop=(kt == KT_D - 1),
                    )
                ob = misc.tile([P, CW], F32, tag="ob", bufs=2, name="ob")
                nc.vector.tensor_copy(ob[:], m[:, 0:CW])
                nc.sync.dma_start(out_t[:, c0 : c0 + CW], ob[:])

            # chunk 0 scores + V projection (JIT on the SWDGE stream)
            ps_v2_cm = tc.tile_pool(name="ps_v2", bufs=1, space="PSUM")
            ps_v2 = ps_v2_cm.__enter__()
            vacc = {
                half: ps_v2.tile(
                    [P, 1024], F32, tag=f"va{half}", name=f"va{half}"
                )
                for half in range(2)
            }
            exs = {}
            for tt in range(TT):
                exs[(0, tt)] = scores_tt(0, tt)
                dt, half = tt // 2, tt % 2
                for nn in range(2):
                    off = half * 1024 + nn * NQ
                    nc.tensor.matmul(
                        vacc[half][:, nn * NQ : (nn + 1) * NQ],
                        wtiles["wv"][:, dt, :],
                        ev_tiles[dt][:, off : off + NQ],
                        start=(dt == 0),
                        stop=(dt == KT_D - 1),
                    )

            # chunk 1: vt copies + transposes, then scores + ctx(0) drain
            ps_cx_cm = None
            qq_t = {}
            for tt in range(TT):
                if tt == 0:
                    for nn in range(4):
                        nc.vector.tensor_copy(
                            vt_sb[:, nn * NQ : (nn + 1) * NQ],
                            vacc[nn // 2][:, (nn % 2) * NQ : (nn % 2 + 1) * NQ],
                        )
                ex = scores_tt(1, tt)
                exs[(1, tt)] = ex
                if tt < 8:
                    for j in range(2):
                        k = 2 * tt + j
                        tp = ps_v2.tile(
                            [P, P], BF16, tag="va0", name="tp"
                        )
                        nc.tensor.transpose(
                            tp[:], vt_sb[:, k * P : (k + 1) * P], ident[:]
                        )
                        nc.vector.tensor_copy(v_aug[:, k, 0:DV], tp[:, 0:DV])
                        nc.vector.tensor_copy(
                            v_aug[:, k, DV + 1 : 2 * DV + 1],
                            tp[:, DV : 2 * DV],
                        )
                else:
                    if tt == 8:
                        ps_v2_cm.__exit__(None, None, None)
                        ps_cx_cm = tc.tile_pool(
                            name="ps_cx", bufs=1, space="PSUM"
                        )
                        ps_cx = ps_cx_cm.__enter__()
                        alloc_ctx(0)
                        qq_t[2] = ps_cx.tile(
                            [P, CW], F32, tag="qq23", name="qq2"
                        )
                    ctx_op(0, tt - 8, exs[(0, tt - 8)])
                    dt = tt - 8
                    eq = load_qq(2, dt)
                    nc.tensor.matmul(
                        qq_t[2][:],
                        wtiles["wq"][:, dt, :],
                        eq[:],
                        start=(dt == 0),
                        stop=(dt == KT_D - 1),
                    )

            nc.vector.tensor_copy(qt_sb[:, 2 * CW : 3 * CW], qq_t[2][:])

            # chunks 2..3 + steady ctx drain (lag 8)
            for ci in (2, 3):
                for tt in range(TT):
                    ex = scores_tt(ci, tt)
                    exs[(ci, tt)] = ex
                    if tt < 8:
                        ctx_op(ci - 2, tt + 8, exs[(ci - 2, tt + 8)])
                        if ci == 2 and tt < 6:
                            dt = tt
                            if tt == 0:
                                qq_t[3] = ps_cx.tile(
                                    [P, CW], F32, tag="qq23", name="qq3"
                                )
                            eq = load_qq(3, dt)
                            nc.tensor.matmul(
                                qq_t[3][:],
                                wtiles["wq"][:, dt, :],
                                eq[:],
                                start=(dt == 0),
                                stop=False,
                            )
                    else:
                        if tt == 8:
                            if ci == 2:
                                normalize(0)
                                alloc_ctx(1)
                                for dt in (6, 7):
                                    eq = load_qq(3, dt)
                                    nc.tensor.matmul(
                                        qq_t[3][:],
                                        wtiles["wq"][:, dt, :],
                                        eq[:],
                                        start=False,
                                        stop=(dt == 7),
                                    )
                                nc.vector.tensor_copy(
                                    qt_sb[:, 3 * CW : 4 * CW], qq_t[3][:]
                                )
                            else:
                                normalize(1)
                                alloc_ctx(2)
                        ctx_op(ci - 1, tt - 8, exs[(ci - 1, tt - 8)])

            # tail: remaining ctx + normalizes + outprojs
            for k in range(8, TT):
                ctx_op(2, k, exs[(2, k)])
            normalize(2)
            alloc_ctx(3)
            for k in range(TT):
                ctx_op(3, k, exs[(3, k)])
            normalize(3)
            for ci in range(4):
                outproj(ci)
            ps_cx_cm.__exit__(None, None, None)
            ps_at_cm.__exit__(None, None, None)

    nc.compile()
    return nc


def kernel(
    encodings_for_q,
    encodings_for_k,
    encodings_for_v,
    W_q,
    W_k,
    W_v,
    W_out,
    _trace: bool = False,
):
    encodings_for_q = np.asarray(encodings_for_q, dtype=np.float32)
    encodings_for_k = np.asarray(encodings_for_k, dtype=np.float32)
    encodings_for_v = np.asarray(encodings_for_v, dtype=np.float32)
    W_q = np.asarray(W_q, dtype=np.float32)
    W_k = np.asarray(W_k, dtype=np.float32)
    W_v = np.asarray(W_v, dtype=np.float32)
    W_out = np.asarray(W_out, dtype=np.float32)

    if "nc" not in _cache:
        _cache["nc"] = build()
    nc = _cache["nc"]

    eqT = np.ascontiguousarray(encodings_for_q.T)
    ekT = np.ascontiguousarray(encodings_for_k.T)
    evT = np.ascontiguousarray(encodings_for_v.T)

    in_maps = []
    for c in range(NCORES):
        hs = slice(HPC * c, HPC * (c + 1))
        in_maps.append(
            {
                "encq_t": eqT,
                "enck_t": ekT,
                "encv_t": evT,
                "wq": _prep_w(np.transpose(W_q[hs], (1, 0, 2)).reshape(D, FW)),
                "wk": _prep_w(np.transpose(W_k[hs], (1, 0, 2)).reshape(D, FW)),
                "wv": _prep_w(np.transpose(W_v[hs], (1, 0, 2)).reshape(D, FW)),
                "wo": _prep_w(W_out[:, FW * c : FW * (c + 1)]),
            }
        )

    r = run_bass_kernel_spmd(
        nc, in_maps, core_ids=list(range(NCORES)), trace=_trace
    )
    out = np.concatenate(
        [r.results[c]["outT"].T for c in range(NCORES)], axis=1
    )
    if _trace:
        kernel.last_exec_time_ns = r.exec_time_ns
        kernel.last_insts = (
            r.instructions_and_trace[0] if r.instructions_and_trace else None
        )
    return out.astype(np.float32)


# revision 18
# speedup vs baseline: 1.0824x; 1.0824x over previous
"""Multi-head attention (S=2048, D=1024, H=16, dk=dv=64) on 8 TRN2 NeuronCores.

Sharding: head-parallel tensor parallelism. Core c owns heads {2c, 2c+1}:
  - QT/KT [128, S] (two heads stacked on partitions); V via PE-transpose of
    VT, augmented with a ones column so the ctx matmul also produces the
    softmax denominators (softmax runs over the partition axis).
  - scoresT tiles -> exp on ACT (scale=1/8 folded in) -> ctx accumulation.
  - per s-chunk: normalize ctxT, AllGather the [128, chunk] block across
    cores -> [1024, chunk] concat, then a 128-column slice of the output
    projection per core (outT layout). Host unshard = concat + transpose.

Overlap structure: enc_k/enc_q stream on the two HWDGE queues while enc_v
cast-streams on the SWDGE queue, all from t=0. K and the first half of Q
project first (the minimum needed to start the exp stream); V and the
second Q half project inside the first chunk's scores loop using
time-shared PSUM pools. s is processed in three chunks (1024/512/512) so
the per-chunk ctx AllGathers start early and the last one is small.

Compute dtype: bf16 operands, fp32 PSUM accumulation, softmax in fp32.
"""

import numpy as np

import concourse.bass as bass
import concourse.mybir as mybir
import concourse.tile as tile
from concourse import bacc
from concourse.bass_utils import run_bass_kernel_spmd

S = 2048
D = 1024
H = 16
DK = 64
DV = 64
NCORES = 8
HPC = H // NCORES          # heads per core = 2
FW = HPC * DV              # per-core feature width = 128
P = 128                    # partitions
KT_D = D // P              # 8 contraction tiles over D
TT = S // P                # 16 tiles over t (keys)
NQ = 512                   # matmul moving free dim
CW = 512                   # s-chunk width (ctx/AG granularity)
NC_CH = S // CW            # 4 chunks
VA = 2 * (DV + 1)          # V_aug feature width

F32 = mybir.dt.float32
BF16 = mybir.dt.bfloat16
EXPF = mybir.ActivationFunctionType.Exp

_cache = {}


def _prep_w(w):
    """[D, FW] -> [128, KT_D*FW]: row p holds all d-tiles' row p."""
    return np.ascontiguousarray(
        np.transpose(w.reshape(KT_D, P, FW), (1, 0, 2)).reshape(P, KT_D * FW)
    )



def build():
    nc = bacc.Bacc(None, target_bir_lowering=False)

    enc_in = {
        x: nc.dram_tensor(f"enc{x}_t", [D, S], F32, kind="ExternalInput")
        for x in ("q", "k", "v")
    }
    w_in = {
        n: nc.dram_tensor(n, [P, KT_D * FW], F32, kind="ExternalInput")
        for n in ("wq", "wk", "wv", "wo")
    }
    out_t = nc.dram_tensor("outT", [FW, S], F32, kind="ExternalOutput")

    from concourse.bass import _add_dep_helper
    from concourse.masks import make_identity

    with tile.TileContext(nc) as tc:
        with (
            tc.tile_pool(name="wts", bufs=1) as wts,
            tc.tile_pool(name="encp", bufs=3) as encp,
            tc.tile_pool(name="qkv", bufs=1) as qkv,
            tc.tile_pool(name="expp", bufs=13) as expp,
            tc.tile_pool(name="catp", bufs=1) as catp,
            tc.tile_pool(name="catin", bufs=3) as catin,
            tc.tile_pool(name="misc", bufs=1) as misc,
            tc.tile_pool(name="dram", bufs=1, space="DRAM") as dram,
        ):
            rg = [list(range(NCORES))]

            # ---- weights: contiguous f32 DMA + DVE cast ----
            wtiles = {}
            for name in ("wq", "wk", "wv", "wo"):
                wraw = encp.tile(
                    [P, KT_D * FW], F32, tag="rawwt", bufs=2, name=name
                )
                nc.sync.dma_start(wraw[:], w_in[name][:])
                wt = wts.tile([P, KT_D, FW], BF16, tag=f"w_{name}", name=name)
                nc.vector.tensor_copy(
                    wt[:], wraw.rearrange("p (kt m) -> p kt m", kt=KT_D)
                )
                wtiles[name] = wt

            ident = wts.tile([P, P], BF16, tag="ident")
            make_identity(nc, ident)

            # persistent SBUF state
            qt_sb = qkv.tile([P, S], BF16, tag="qt")
            kt_sb = qkv.tile([P, S], BF16, tag="kt")
            vt_sb = qkv.tile([P, S], BF16, tag="vt")
            v_aug = qkv.tile([P, TT, VA], BF16, tag="vaug")
            cat_loc = catp.tile([P, S], BF16, tag="cat")
            nc.any.memset(v_aug[:, :, DV : DV + 1], 1.0)
            nc.any.memset(v_aug[:, :, 2 * DV + 1 : 2 * DV + 2], 1.0)

            # K tiles: halves on the two HWDGE queues, DVE cast
            def load_k(dt):
                raw = encp.tile([P, S], F32, tag="rawk", bufs=3, name="raw")
                nc.sync.dma_start(
                    raw[:, :1024], enc_in["k"][dt * P : (dt + 1) * P, 0:1024]
                )
                nc.scalar.dma_start(
                    raw[:, 1024:], enc_in["k"][dt * P : (dt + 1) * P, 1024:]
                )
                t = encp.tile([P, S], BF16, tag="bfk", bufs=5, name="bf")
                nc.vector.tensor_copy(t[:], raw[:])
                return t

            # Q quarter tiles [128, 512], alternating queues, DVE cast
            gate_inst = [None]

            def load_qq(qq, dt):
                c0 = qq * CW
                raw = encp.tile([P, CW], F32, tag="rawq", bufs=6, name="raw")
                eng = nc.sync if dt % 2 == 0 else nc.scalar
                d = eng.dma_start(
                    raw[:], enc_in["q"][dt * P : (dt + 1) * P, c0 : c0 + CW]
                )
                if qq == 1 and dt == 4:
                    gate_inst[0] = d.ins
                t = encp.tile([P, CW], BF16, tag="bfq", bufs=18, name="bf")
                nc.vector.tensor_copy(t[:], raw[:])
                return t

            # ---- phase 0: K, Q quarters 0+1; PE warm-up burst ----
            ps_p_cm = tc.tile_pool(name="ps_p", bufs=1, space="PSUM")
            ps_p = ps_p_cm.__enter__()
            kacc = {
                sc4: ps_p.tile([P, NQ], F32, tag=f"ka{sc4}", name=f"ka{sc4}")
                for sc4 in range(4)
            }
            ek0 = load_k(0)
            wm = ps_p.tile([P, NQ], F32, tag="warm", name="wm")
            for _ in range(24):
                nc.tensor.matmul(
                    wm[:], wtiles["wk"][:, 0, :], ek0[:, 0:NQ],
                    start=True, stop=True,
                )
            for dt in range(KT_D):
                ek = ek0 if dt == 0 else load_k(dt)
                for sc4 in range(4):
                    nc.tensor.matmul(
                        kacc[sc4][:],
                        wtiles["wk"][:, dt, :],
                        ek[:, sc4 * NQ : (sc4 + 1) * NQ],
                        start=(dt == 0),
                        stop=(dt == KT_D - 1),
                    )
            for sc4 in range(4):
                nc.vector.tensor_copy(
                    kt_sb[:, sc4 * NQ : (sc4 + 1) * NQ], kacc[sc4][:]
                )
            qq01 = {
                qq: ps_p.tile([P, CW], F32, tag=f"qq{qq}", name=f"qq{qq}")
                for qq in range(2)
            }
            last_q_dma = [None]
            for qq in range(2):
                for dt in range(KT_D):
                    eq = load_qq(qq, dt)
                    nc.tensor.matmul(
                        qq01[qq][:],
                        wtiles["wq"][:, dt, :],
                        eq[:],
                        start=(dt == 0),
                        stop=(dt == KT_D - 1),
                    )
                nc.vector.tensor_copy(
                    qt_sb[:, qq * CW : (qq + 1) * CW], qq01[qq][:]
                )
            ps_p_cm.__exit__(None, None, None)

            # ---- enc_v: SWDGE cast-DMA stream, held behind the K/Q
            # stream via an artificial dep on the last Q-quarter-1 tile ----
            from concourse.bass import _add_dep_helper

            ev_tiles = []
            for dt in range(KT_D):
                ev = encp.tile([P, S], BF16, tag="encv", bufs=6, name="ev")
                d = nc.gpsimd.dma_start(
                    ev[:], enc_in["v"][dt * P : (dt + 1) * P, :]
                )
                if dt == 0 and gate_inst[0] is not None:
                    _add_dep_helper(d.ins, gate_inst[0], sync=True,
                                    reason="defer enc_v behind K/Q stream")
                ev_tiles.append(ev)

            qq_pre = {}
            for qq in (2, 3):
                for dt in range(KT_D):
                    qq_pre[(qq, dt)] = load_qq(qq, dt)

            # ---- attention stream ----
            ps_at_cm = tc.tile_pool(name="ps_at", bufs=1, space="PSUM")
            ps_at = ps_at_cm.__enter__()
            ctx_ps = {}
            gas = {}

            def scores_tt(ci, tt):
                m = ps_at.tile([P, 1024], F32, tag="mega", bufs=2, name="m")
                s0 = ci * CW
                for h in range(HPC):
                    nc.tensor.matmul(
                        m[:, h * NQ : (h + 1) * NQ],
                        kt_sb[h * DK : (h + 1) * DK, tt * P : (tt + 1) * P],
                        qt_sb[h * DK : (h + 1) * DK, s0 : s0 + NQ],
                        start=True,
                        stop=True,
                    )
                ex = expp.tile(
                    [P, 1024], BF16, tag=f"exp{tt % 2}", bufs=13, name="ex"
                )
                nc.scalar.activation(ex[:], m[:], EXPF, scale=1.0 / np.sqrt(DK))
                return ex

            def ctx_op(ci, k, ex):
                for h in range(HPC):
                    nc.tensor.matmul(
                        ctx_ps[ci][h * DV + (h > 0) : (h + 1) * DV + 1, :]
                        if False
                        else ctx_ps[(ci, h)][:, :],
                        v_aug[:, k, h * (DV + 1) : (h + 1) * (DV + 1)],
                        ex[:, h * NQ : (h + 1) * NQ],
                        start=(k == 0),
                        stop=(k == TT - 1),
                    )

            def alloc_ctx(ci):
                for h in range(HPC):
                    ctx_ps[(ci, h)] = ps_cx.tile(
                        [DV + 1, CW], F32, tag=f"cx{h}", name=f"cx{ci}{h}"
                    )

            def normalize(ci):
                c0 = ci * CW
                for h in range(HPC):
                    den = misc.tile([1, CW], F32, tag="den", name="den")
                    nc.vector.tensor_copy(
                        den[:], ctx_ps[(ci, h)][DV : DV + 1, :]
                    )
                    recip = misc.tile([1, CW], F32, tag="recip", name="recip")
                    nc.vector.reciprocal_approx_fast(recip[:], den[:])
                    bcast = misc.tile([DV, CW], F32, tag="bcast", name="bcast")
                    nc.gpsimd.partition_broadcast(bcast[:], recip[:])
                    nc.vector.tensor_mul(
                        cat_loc[h * DV : (h + 1) * DV, c0 : c0 + CW],
                        ctx_ps[(ci, h)][0:DV, :],
                        bcast[:],
                    )
                cb = dram.tile([P, CW], BF16, tag=f"catb{ci}", name="cb")
                nc.sync.dma_start(cb[:], cat_loc[:, c0 : c0 + CW])
                ga = dram.tile([D, CW], BF16, tag=f"catall{ci}", name="ga")
                nc.gpsimd.collective_compute(
                    "AllGather",
                    mybir.AluOpType.bypass,
                    ins=[cb[:].opt()],
                    outs=[ga[:].opt()],
                    replica_groups=rg,
                )
                gas[ci] = ga

            def outproj(ci):
                c0 = ci * CW
                m = ps_at.tile([P, 1024], F32, tag="mega", bufs=2, name="om")
                for kt in range(KT_D):
                    ct = catin.tile([P, CW], BF16, tag="catkt", name="ct")
                    nc.sync.dma_start(
                        ct[:], gas[ci][kt * P : (kt + 1) * P, :]
                    )
                    nc.tensor.matmul(
                        m[:, 0:CW],
                        wtiles["wo"][:, kt, :],
                        ct[:],
                        start=(kt == 0),
                        stop=(kt == KT_D - 1),
                    )
                ob = misc.tile([P, CW], F32, tag="ob", bufs=2, name="ob")
                nc.vector.tensor_copy(ob[:], m[:, 0:CW])
                nc.sync.dma_start(out_t[:, c0 : c0 + CW], ob[:])

            # chunk 0 scores + V projection (JIT on the SWDGE stream)
            ps_v2_cm = tc.tile_pool(name="ps_v2", bufs=1, space="PSUM")
            ps_v2 = ps_v2_cm.__enter__()
            vacc = {
                half: ps_v2.tile(
                    [P, 1024], F32, tag=f"va{half}", name=f"va{half}"
                )
                for half in range(2)
            }
            exs = {}
            for tt in range(TT):
                exs[(0, tt)] = scores_tt(0, tt)
                dt, half = tt // 2, tt % 2
                for nn in range(2):
                    off = half * 1024 + nn * NQ
                    nc.tensor.matmul(
                        vacc[half][:, nn * NQ : (nn + 1) * NQ],
                        wtiles["wv"][:, dt, :],
                        ev_tiles[dt][:, off : off + NQ],
                        start=(dt == 0),
                        stop=(dt == KT_D - 1),
                    )

            # chunk 1: vt copies + transposes, then scores + ctx(0) drain
            ps_cx_cm = None
            qq_t = {}
            for tt in range(TT):
                if tt == 0:
                    for nn in range(4):
                        nc.vector.tensor_copy(
                            vt_sb[:, nn * NQ : (nn + 1) * NQ],
                            vacc[nn // 2][:, (nn % 2) * NQ : (nn % 2 + 1) * NQ],
                        )
                ex = scores_tt(1, tt)
                exs[(1, tt)] = ex
                if tt < 8:
                    for j in range(2):
                        k = 2 * tt + j
                        tp = ps_v2.tile(
                            [P, P], BF16, tag="va0", name="tp"
                        )
                        nc.tensor.transpose(
                            tp[:], vt_sb[:, k * P : (k + 1) * P], ident[:]
                        )
                        nc.vector.tensor_copy(v_aug[:, k, 0:DV], tp[:, 0:DV])
                        nc.vector.tensor_copy(
                            v_aug[:, k, DV + 1 : 2 * DV + 1],
                            tp[:, DV : 2 * DV],
                        )
                else:
                    if tt == 8:
                        ps_v2_cm.__exit__(None, None, None)
                        ps_cx_cm = tc.tile_pool(
                            name="ps_cx", bufs=1, space="PSUM"
                        )
                        ps_cx = ps_cx_cm.__enter__()
                        alloc_ctx(0)
                        qq_t[2] = ps_cx.tile(
                            [P, CW], F32, tag="qq23", name="qq2"
                        )
                    ctx_op(0, tt - 8, exs[(0, tt - 8)])
                    dt = tt - 8
                    eq = qq_pre[(2, dt)]
                    nc.tensor.matmul(
                        qq_t[2][:],
                        wtiles["wq"][:, dt, :],
                        eq[:],
                        start=(dt == 0),
                        stop=(dt == KT_D - 1),
                    )

            nc.vector.tensor_copy(qt_sb[:, 2 * CW : 3 * CW], qq_t[2][:])

            # chunks 2..3 + steady ctx drain (lag 8)
            for ci in (2, 3):
                for tt in range(TT):
                    ex = scores_tt(ci, tt)
                    exs[(ci, tt)] = ex
                    if tt < 8:
                        ctx_op(ci - 2, tt + 8, exs[(ci - 2, tt + 8)])
                        if ci == 2 and tt < 6:
                            dt = tt
                            if tt == 0:
                                qq_t[3] = ps_cx.tile(
                                    [P, CW], F32, tag="qq23", name="qq3"
                                )
                            eq = qq_pre[(3, dt)]
                            nc.tensor.matmul(
                                qq_t[3][:],
                                wtiles["wq"][:, dt, :],
                                eq[:],
                                start=(dt == 0),
                                stop=False,
                            )
                    else:
                        if tt == 8:
                            if ci == 2:
                                normalize(0)
                                alloc_ctx(1)
                                for dt in (6, 7):
                                    eq = qq_pre[(3, dt)]
                                    nc.tensor.matmul(
                                        qq_t[3][:],
                                        wtiles["wq"][:, dt, :],
                                        eq[:],
                                        start=False,
                                        stop=(dt == 7),
                                    )
                                nc.vector.tensor_copy(
                                    qt_sb[:, 3 * CW : 4 * CW], qq_t[3][:]
                                )
                            else:
                                normalize(1)
                                alloc_ctx(2)
                        ctx_op(ci - 1, tt - 8, exs[(ci - 1, tt - 8)])

            # tail: remaining ctx + normalizes + outprojs
            for k in range(8, TT):
                ctx_op(2, k, exs[(2, k)])
            normalize(2)
            alloc_ctx(3)
            for k in range(TT):
                ctx_op(3, k, exs[(3, k)])
            normalize(3)
            for ci in range(4):
                outproj(ci)
            ps_cx_cm.__exit__(None, None, None)
            ps_at_cm.__exit__(None, None, None)

    nc.compile()
    return nc


def kernel(
    encodings_for_q,
    encodings_for_k,
    encodings_for_v,
    W_q,
    W_k,
    W_v,
    W_out,
    _trace: bool = False,
):
    encodings_for_q = np.asarray(encodings_for_q, dtype=np.float32)
    encodings_for_k = np.asarray(encodings_for_k, dtype=np.float32)
    encodings_for_v = np.asarray(encodings_for_v, dtype=np.float32)
    W_q = np.asarray(W_q, dtype=np.float32)
    W_k = np.asarray(W_k, dtype=np.float32)
    W_v = np.asarray(W_v, dtype=np.float32)
    W_out = np.asarray(W_out, dtype=np.float32)

    if "nc" not in _cache:
        _cache["nc"] = build()
    nc = _cache["nc"]

    eqT = np.ascontiguousarray(encodings_for_q.T)
    ekT = np.ascontiguousarray(encodings_for_k.T)
    evT = np.ascontiguousarray(encodings_for_v.T)

    in_maps = []
    for c in range(NCORES):
        hs = slice(HPC * c, HPC * (c + 1))
        in_maps.append(
            {
                "encq_t": eqT,
                "enck_t": ekT,
                "encv_t": evT,
                "wq": _prep_w(np.transpose(W_q[hs], (1, 0, 2)).reshape(D, FW)),
                "wk": _prep_w(np.transpose(W_k[hs], (1, 0, 2)).reshape(D, FW)),
                "wv": _prep_w(np.transpose(W_v[hs], (1, 0, 2)).reshape(D, FW)),
                "wo": _prep_w(W_out[:, FW * c : FW * (c + 1)]),
            }
        )

    r = run_bass_kernel_spmd(
        nc, in_maps, core_ids=list(range(NCORES)), trace=_trace
    )
    out = np.concatenate(
        [r.results[c]["outT"].T for c in range(NCORES)], axis=1
    )
    if _trace:
        kernel.last_exec_time_ns = r.exec_time_ns
        kernel.last_insts = (
            r.instructions_and_trace[0] if r.instructions_and_trace else None
        )
    return out.astype(np.float32)
